# revision 8
# baseline (speedup 1.0000x reference)
"""LocallyConnected2d (3x3, 64x64 out, C_in=16, C_out=32, B=32) on 8 trn2 cores.

out[b,o,h,w] = sum_{c,i,j} x[b,c,h+i,w+j] * weight[0,o,c,h,w,(i,j)] + bias[0,o,h,w]

Sharding: spatial over H_out - core i computes output rows 8i..8i+8, needing
input rows 8i..8i+10 (halo) and its 1/8 slice of the (per-position, unique)
weights. Weights dominate traffic (75MB total) and are read exactly once.

Per position (h,w): one K=145 contraction (9 taps x 16 ch + 1 bias row),
M=32 (C_out, stationary), N=32 (batch, moving), split K=128 + K=17 with PSUM
accumulation. The bias is folded into the weight tensor as a 145th
contraction row against a constant-1.0 activation row.

x is staged in SBUF as 9 shifted replicas (one per kernel tap (i,j), each
DMA'd from DRAM with flat offset 66*i+j baked in), so the moving operand for
every position is a uniform strided slice - no im2col step, no per-position
copies.
"""

import numpy as np

import concourse.bass as bass
import concourse.mybir as mybir
import concourse.tile as tile
from concourse import bacc
from concourse import bass_utils

N_CORES = 8
B, CI, CO = 32, 16, 32
H = W = 64
HL = H // N_CORES          # output rows per core
XROWS = HL + 2             # input rows per core (with halo)
XW = 66
XFLAT = XROWS * XW         # 660
T = (HL - 1) * XW + W      # 526: flat window covering all (h,w) offsets
KA, KB = 128, 17           # contraction split: 8 taps x 16ch | 1 tap x 16ch + bias
NG = 4                     # w-groups per row
GW = W // NG               # 16 positions per group

USE_BF16 = False

_cache = {}


def _np_dt(use_bf16):
    if use_bf16:
        import ml_dtypes
        return np.dtype(ml_dtypes.bfloat16)
    return np.dtype(np.float32)


def _build(use_bf16, n_iters=1):
    dt = mybir.dt.bfloat16 if use_bf16 else mybir.dt.float32
    f32 = mybir.dt.float32
    nc = bacc.Bacc("TRN2", target_bir_lowering=False, debug=False,
                   num_devices=N_CORES)
    xs_d = nc.dram_tensor("xs", [CI + 1, B, XFLAT], dt, kind="ExternalInput")
    ws_d = nc.dram_tensor("ws", [HL, NG, KA + KB, GW, CO], dt,
                          kind="ExternalInput")
    out_d = nc.dram_tensor("out", [HL, CO, W, B], f32, kind="ExternalOutput")

    with tile.TileContext(nc) as tc:
        with (
            tc.tile_pool(name="px", bufs=1) as px,
            tc.tile_pool(name="pw", bufs=4) as pw,
            tc.tile_pool(name="po", bufs=2) as po,
            tc.tile_pool(name="pp", bufs=4, space=bass.MemorySpace.PSUM) as pp,
        ):
            for _ in range(n_iters):
                # 9 shifted replicas of x + a ones row (bias multiplier)
                pa = px.tile([KA, B, T], dt, tag="pa")
                pb = px.tile([KB, B, T], dt, tag="pb")
                for k in range(8):
                    i, j = divmod(k, 3)
                    off = XW * i + j
                    nc.sync.dma_start(pa[16 * k:16 * (k + 1), :, :],
                                      xs_d[0:CI, :, off:off + T])
                nc.sync.dma_start(pb[0:16, :, :],
                                  xs_d[0:CI, :, 2 * XW + 2:2 * XW + 2 + T])
                nc.sync.dma_start(pb[16:17, :, :], xs_d[CI:CI + 1, :, 0:T])

                for h in range(HL):
                    out_sb = po.tile([CO, W, B], f32, tag="osb")
                    for g in range(NG):
                        wa = pw.tile([KA, GW, CO], dt, tag="wa")
                        wb = pw.tile([KB, GW, CO], dt, tag="wb")
                        nc.sync.dma_start(wa[:], ws_d[h, g, 0:KA, :, :])
                        nc.sync.dma_start(wb[:], ws_d[h, g, KA:KA + KB, :, :])
                        ps = pp.tile([CO, GW, B], f32, tag="ps")
                        for wl in range(GW):
                            t = XW * h + g * GW + wl
                            nc.tensor.matmul(ps[:, wl, :], wa[:, wl, :],
                                             pa[:, :, t], start=True, stop=False)
                            nc.tensor.matmul(ps[:, wl, :], wb[:, wl, :],
                                             pb[:, :, t], start=False, stop=True)
                        nc.vector.tensor_copy(out_sb[:, g * GW:(g + 1) * GW, :],
                                              ps[:])
                    nc.sync.dma_start(out_d[h], out_sb[:])
    nc.compile()
    return nc


def _get_nc(use_bf16, n_iters=1):
    key = (use_bf16, n_iters)
    if key not in _cache:
        _cache[key] = _build(use_bf16, n_iters)
    return _cache[key]


def _pack_inputs(x, weight, bias, use_bf16):
    """Full inputs -> per-core in_maps (host-side shard + relayout)."""
    np_dt = _np_dt(use_bf16)
    x = np.asarray(x, np.float32)
    weight = np.asarray(weight, np.float32)
    bias = np.asarray(bias, np.float32)

    # weights: [1,o,c,h,w,k] -> [h, w, k=(tap,kc), o], bias appended as k=144
    wt = weight[0].transpose(2, 3, 4, 1, 0).reshape(H, W, 9 * CI, CO)
    bt = bias[0].transpose(1, 2, 0)[:, :, None, :]          # [h, w, 1, o]
    wfull = np.concatenate([wt, bt], axis=2)                # [h, w, 145, o]

    in_maps = []
    for c in range(N_CORES):
        r0 = HL * c
        xs = x[:, :, r0:r0 + XROWS, :].transpose(1, 0, 2, 3).reshape(CI, B, XFLAT)
        xs = np.concatenate([xs, np.ones((1, B, XFLAT), np.float32)], axis=0)
        xs = np.ascontiguousarray(xs, dtype=np_dt)
        wc = wfull[r0:r0 + HL].reshape(HL, NG, GW, KA + KB, CO)
        wc = np.ascontiguousarray(wc.transpose(0, 1, 3, 2, 4), dtype=np_dt)
        in_maps.append({"xs": xs, "ws": wc})
    return in_maps


def _gather(results):
    outs = np.stack([results[c]["out"] for c in range(N_CORES)])
    out = outs.reshape(H, CO, W, B).transpose(3, 1, 0, 2)   # -> [b, o, h, w]
    return np.ascontiguousarray(out)


def run(x, weight, bias, use_bf16=None, n_iters=1, **spmd_kwargs):
    if use_bf16 is None:
        use_bf16 = USE_BF16
    nc = _get_nc(use_bf16, n_iters)
    in_maps = _pack_inputs(x, weight, bias, use_bf16)
    res = bass_utils.run_bass_kernel_spmd(nc, in_maps,
                                          core_ids=list(range(N_CORES)),
                                          **spmd_kwargs)
    return _gather(res.results), res


def kernel(x, weight, bias):
    out, _ = run(x, weight, bias)
    return out


# revision 9
# speedup vs baseline: 344.9676x; 344.9676x over previous
"""LocallyConnected2d (3x3, 64x64 out, C_in=16, C_out=32, B=32) on 8 trn2 cores.

out[b,o,h,w] = sum_{c,i,j} x[b,c,h+i,w+j] * weight[0,o,c,h,w,(i,j)] + bias[0,o,h,w]

Sharding: spatial over H_out - core i computes output rows 8i..8i+8, needing
input rows 8i..8i+10 (halo) and its 1/8 slice of the (per-position, unique)
weights. Weights dominate traffic (75MB total) and are read exactly once.

Per position (h,w): one K=145 contraction (9 taps x 16 ch + 1 bias row),
M=32 (C_out, stationary), N=32 (batch, moving), split K=128 + K=17 with PSUM
accumulation. The bias is folded into the weight tensor as a 145th
contraction row against a constant-1.0 activation row.

x is staged in SBUF as 9 shifted replicas (one per kernel tap (i,j), each
DMA'd from DRAM with flat offset 66*i+j baked in), so the moving operand for
every position is a uniform strided slice - no im2col step, no per-position
copies.
"""

import numpy as np

import concourse.bass as bass
import concourse.mybir as mybir
import concourse.tile as tile
from concourse import bacc
from concourse import bass_utils

N_CORES = 8
B, CI, CO = 32, 16, 32
H = W = 64
HL = H // N_CORES          # output rows per core
XROWS = HL + 2             # input rows per core (with halo)
XW = 66
XFLAT = XROWS * XW         # 660
T = (HL - 1) * XW + W      # 526: flat window covering all (h,w) offsets
KA, KB = 128, 17           # contraction split: 8 taps x 16ch | 1 tap x 16ch + bias
NG = 4                     # w-groups per row
GW = W // NG               # 16 positions per group

USE_BF16 = False

_cache = {}


def _np_dt(use_bf16):
    if use_bf16:
        import ml_dtypes
        return np.dtype(ml_dtypes.bfloat16)
    return np.dtype(np.float32)


def _build(use_bf16, n_iters=1):
    dt = mybir.dt.bfloat16 if use_bf16 else mybir.dt.float32
    f32 = mybir.dt.float32
    nc = bacc.Bacc("TRN2", target_bir_lowering=False, debug=False,
                   num_devices=N_CORES)
    xs_d = nc.dram_tensor("xs", [CI + 1, B, XFLAT], dt, kind="ExternalInput")
    ws_d = nc.dram_tensor("ws", [HL, NG, KA + KB, GW, CO], dt,
                          kind="ExternalInput")
    out_d = nc.dram_tensor("out", [HL, CO, W, B], f32, kind="ExternalOutput")

    import contextlib

    with tile.TileContext(nc) as tc:
        with (
            tc.tile_pool(name="px", bufs=1) as px,
            tc.tile_pool(name="pw", bufs=4) as pw,
            tc.tile_pool(name="po", bufs=2) as po,
            tc.tile_pool(name="pp", bufs=4, space=bass.MemorySpace.PSUM) as pp,
        ):
            loop = (tc.For_i(0, n_iters, 1) if n_iters > 1
                    else contextlib.nullcontext())
            with loop:
                # 9 shifted replicas of x + a ones row (bias multiplier)
                pa = px.tile([KA, B, T], dt, tag="pa")
                pb = px.tile([KB, B, T], dt, tag="pb")
                for k in range(8):
                    i, j = divmod(k, 3)
                    off = XW * i + j
                    nc.sync.dma_start(pa[16 * k:16 * (k + 1), :, :],
                                      xs_d[0:CI, :, off:off + T])
                nc.sync.dma_start(pb[0:16, :, :],
                                  xs_d[0:CI, :, 2 * XW + 2:2 * XW + 2 + T])
                nc.sync.dma_start(pb[16:17, :, :], xs_d[CI:CI + 1, :, 0:T])

                for h in range(HL):
                    out_sb = po.tile([CO, W, B], f32, tag="osb")
                    for g in range(NG):
                        wa = pw.tile([KA, GW, CO], dt, tag="wa")
                        wb = pw.tile([KB, GW, CO], dt, tag="wb")
                        nc.sync.dma_start(wa[:], ws_d[h, g, 0:KA, :, :])
                        nc.sync.dma_start(wb[:], ws_d[h, g, KA:KA + KB, :, :])
                        ps = pp.tile([CO, GW, B], f32, tag="ps")
                        for wl in range(GW):
                            t = XW * h + g * GW + wl
                            nc.tensor.matmul(ps[:, wl, :], wa[:, wl, :],
                                             pa[:, :, t], start=True, stop=False)
                            nc.tensor.matmul(ps[:, wl, :], wb[:, wl, :],
                                             pb[:, :, t], start=False, stop=True)
                        nc.vector.tensor_copy(out_sb[:, g * GW:(g + 1) * GW, :],
                                              ps[:])
                    nc.sync.dma_start(out_d[h], out_sb[:])
    nc.compile()
    return nc


def _get_nc(use_bf16, n_iters=1):
    key = (use_bf16, n_iters)
    if key not in _cache:
        _cache[key] = _build(use_bf16, n_iters)
    return _cache[key]


def _pack_inputs(x, weight, bias, use_bf16):
    """Full inputs -> per-core in_maps (host-side shard + relayout)."""
    np_dt = _np_dt(use_bf16)
    x = np.asarray(x, np.float32)
    weight = np.asarray(weight, np.float32)
    bias = np.asarray(bias, np.float32)

    # weights: [1,o,c,h,w,k] -> [h, w, k=(tap,kc), o], bias appended as k=144
    wt = weight[0].transpose(2, 3, 4, 1, 0).reshape(H, W, 9 * CI, CO)
    bt = bias[0].transpose(1, 2, 0)[:, :, None, :]          # [h, w, 1, o]
    wfull = np.concatenate([wt, bt], axis=2)                # [h, w, 145, o]

    in_maps = []
    for c in range(N_CORES):
        r0 = HL * c
        xs = x[:, :, r0:r0 + XROWS, :].transpose(1, 0, 2, 3).reshape(CI, B, XFLAT)
        xs = np.concatenate([xs, np.ones((1, B, XFLAT), np.float32)], axis=0)
        xs = np.ascontiguousarray(xs, dtype=np_dt)
        wc = wfull[r0:r0 + HL].reshape(HL, NG, GW, KA + KB, CO)
        wc = np.ascontiguousarray(wc.transpose(0, 1, 3, 2, 4), dtype=np_dt)
        in_maps.append({"xs": xs, "ws": wc})
    return in_maps


def _gather(results):
    outs = np.stack([results[c]["out"] for c in range(N_CORES)])
    out = outs.reshape(H, CO, W, B).transpose(3, 1, 0, 2)   # -> [b, o, h, w]
    return np.ascontiguousarray(out)


def run(x, weight, bias, use_bf16=None, n_iters=1, **spmd_kwargs):
    if use_bf16 is None:
        use_bf16 = USE_BF16
    nc = _get_nc(use_bf16, n_iters)
    in_maps = _pack_inputs(x, weight, bias, use_bf16)
    res = bass_utils.run_bass_kernel_spmd(nc, in_maps,
                                          core_ids=list(range(N_CORES)),
                                          **spmd_kwargs)
    return _gather(res.results), res


def kernel(x, weight, bias):
    out, _ = run(x, weight, bias)
    return out
